# revision 43
# baseline (speedup 1.0000x reference)
"""Trainium2 Bass kernel for nn_KLDLoss_18769007083961.

Math reformulation (validated vs reference, rel err ~3e-5 with bf16):
  For each image b, prototype a with class c(a), define over pixels p:
    s_a[p]  = d_a[p] + (label[p] != c(a)) * (-1e4)      # masked-biased distance
    em_a[p] = exp(s_a[p])                               # exactly 0 off-class (underflow)
    Z_a     = sum_p em_a[p]
    G[a,j]  = sum_p em_a[p] * s_j[p]   (j in same group => same class mask)
    A[a,j]  = G[a,j] / Z_a
  Symmetric KL for a same-group pair (i,j) (log-partition terms cancel):
    kld = 0.5 * (A[j,j] - A[j,i] + A[i,i] - A[i,j])
  loss = mean over valid pairs (class count >= 2) of exp(-kld).

Only same-class G entries are consumed, and em is exactly zero off-class,
so the bf16 rounding of the -1e4 bias in s never reaches the result: the
biased s tile can be written once in bf16 and feed both the exp (ACT) and
the matmul lhsT (PE).  bf16 matmul runs at 1 cycle/row vs fp32's 4.

Device kernel (one image per NeuronCore, 8 cores):
  Layout: pixel p = 512*q + 128*w + i  (q = SBUF partition, w = window, i = inner).
  Protos are permuted host-side to class-major order (slot = 8c + 4s + m) so the
  class bias applies to 8 contiguous protos per DVE op.
  Per window: DMA dist -> s_t[128, 80*128] f32; DVE builds (label != c) and
  writes s16[128, 81*128] bf16 = d + mne*-1e4 (slot 80 memset to 1.0);
  ACT computes em = exp(s16) bf16; 128 matmuls (lhsT = s16-slice [128,81],
  rhs = em-slice [128,80]) accumulate out[j,a] = G[a,j], row 80 = Z, in
  PSUM [81,80] f32.  The last window runs in 4 column chunks to shorten the
  pipeline tail after the final DMA.  Host does the tiny 120-pair combine.
"""

import sys
from contextlib import ExitStack

import numpy as np

sys.path.insert(0, "/opt/trn_rl_repo")

import concourse.bass as bass
import concourse.tile as tile
from concourse import mybir
from concourse.bass_utils import run_bass_kernel_spmd
from concourse.tile import add_dep_helper

B = 8
C = 10
NPROT = 80
NSLOT = NPROT + 1  # 80 protos + ones column for Z
P = 65536
Q = 128          # partitions = coarse pixel blocks of 512
W = 4            # windows per image
FI = 128         # inner pixels per window per partition
F32 = mybir.dt.float32
BF16 = mybir.dt.bfloat16

_NC_CACHE = {}


def build_nc():
    nc = bass.Bass()
    # dist pre-transposed host-side to [w, quarter, q, n, i]: every quarter-
    # window DMA is a plain contiguous copy, so the first STT chunk starts
    # after ~1/16 of the image has landed.
    d_in = nc.dram_tensor(
        "dist", [W * 4 * Q, NPROT * (FI // 4)], F32, kind="ExternalInput"
    )
    # labels [q, 512] packed with the 10 class constants -> cols 512..521
    lab_in = nc.dram_tensor("labcls", [Q, 512 + C], F32, kind="ExternalInput")
    g_out = nc.dram_tensor("g", [NSLOT, NPROT], F32, kind="ExternalOutput")

    with ExitStack() as ctx:
        tc = ctx.enter_context(tile.TileContext(nc))
        singles = ctx.enter_context(tc.tile_pool(name="singles", bufs=1))
        spool = ctx.enter_context(tc.tile_pool(name="spool", bufs=2))
        s16pool = ctx.enter_context(tc.tile_pool(name="s16pool", bufs=2))
        empool = ctx.enter_context(tc.tile_pool(name="empool", bufs=2))
        mpool = ctx.enter_context(tc.tile_pool(name="mpool", bufs=2))
        psum = ctx.enter_context(tc.tile_pool(name="psum", bufs=1, space="PSUM"))

        labels_t = singles.tile([Q, 512 + C], F32)
        nc.sync.dma_start(out=labels_t, in_=lab_in[:, :])
        cls_t = labels_t[:, 512 : 512 + C]

        g_ps = psum.tile([NSLOT, NPROT], F32)

        QW_ = FI // 4  # pixels per quarter-window
        QB = NPROT * QW_  # sbuf columns per quarter block
        s_tiles = []
        for w in range(2):
            s_w = spool.tile([Q, NPROT * FI], F32, tag="s", name=f"s_t{w}")
            for k in range(4):
                nc.sync.dma_start(
                    out=s_w[:, k * QB : (k + 1) * QB],
                    in_=d_in[(4 * w + k) * Q : (4 * w + k + 1) * Q, :],
                )
            s_tiles.append(s_w)

        # constant source for the ACT-side absorber writes below
        zconst = singles.tile([Q, 1], BF16)
        nc.vector.memset(zconst, 0.0)

        # Engines have a single sync-wait slot per instruction.  Every
        # cross-engine dependency is therefore carried by a dedicated
        # 1-element absorber op, pinned ahead of its consumer with no-sync
        # dep edges so the scheduler keeps the elision-enabling order.
        first = True
        em_tiles = []
        dead4_tiles = []
        dead4_insts = []
        for w in range(W):
            s_t = s_tiles[w]

            # mne[p, c, i] = (labels != c) as 1.0/0.0, bf16
            mne = mpool.tile([Q, C * FI], BF16, tag="mne")
            mne_v = mne.rearrange("p (c i) -> p c i", c=C)
            lab_w = labels_t[:, w * FI : (w + 1) * FI]
            nc.vector.tensor_tensor(
                mne_v,
                lab_w.unsqueeze(1).broadcast_to([Q, C, FI]),
                cls_t.unsqueeze(2).broadcast_to([Q, C, FI]),
                mybir.AluOpType.not_equal,
            )

            # DVE absorber chain: (1) dist-DMA completion for this window
            probe = mpool.tile([Q, 1], F32, tag="probe", bufs=4)
            i_probe = nc.vector.tensor_copy(probe, s_t[:, 0:1])
            dve_prev = i_probe
            if w >= 2:
                # (2) ACT finished exp(w-2) (read byte from its LAST chunk),
                # which read the s16 buffer the STTs below recycle
                probe2 = mpool.tile([Q, 1], BF16, tag="probe2", bufs=4)
                i_probe2 = nc.vector.tensor_copy(
                    probe2,
                    em_tiles[w - 2][:, (FI - 1) * NPROT : (FI - 1) * NPROT + 1],
                )
                add_dep_helper(i_probe2.ins, dve_prev.ins, sync=False)
                dve_prev = i_probe2

            s16 = s16pool.tile([Q, NSLOT * FI], BF16, tag="s16")
            s16_v = s16.rearrange("p (i n) -> p i n", n=NSLOT)
            em = empool.tile([Q, NPROT * FI], BF16, tag="em")
            em_v = em.rearrange("p (i n) -> p i n", n=NPROT)
            em_tiles.append(em)

            # ones column (slot 80) -> Z row of the gram.  For w >= 2 its
            # bytes were read by every LDW of window w-2, so this memset
            # carries exactly the "PE done with window w-2" wait that the
            # STTs would otherwise each need.
            i_memset = nc.vector.memset(s16_v[:, :, NPROT], 1.0)
            add_dep_helper(i_memset.ins, dve_prev.ins, sync=False)
            dve_prev = i_memset

            # ACT absorber: reading an old-em byte absorbs the same-engine
            # WAW tick (exp(w) overwrites exp(w-2)'s output); the PE tick was
            # absorbed by dead_act at the end of window w-2.
            act_abs = None
            if w >= 2:
                # Read a byte exp(w-2)'s LAST chunk wrote: the single wait
                # "ACT >= exp(w-2, k3)" dominates every chunk's WAW below.
                dead3 = mpool.tile([Q, 1], BF16, tag="dead3", bufs=2)
                act_abs = nc.scalar.copy(
                    dead3,
                    em_tiles[w - 2][
                        :, (FI - 1) * NPROT + 1 : (FI - 1) * NPROT + 2
                    ],
                )
                add_dep_helper(act_abs.ins, dead4_insts[w - 2].ins, sync=False)

            # Chunked pipeline: fine chunks on the first/last window for a
            # fast start/short tail, coarser in the middle to cut DVE
            # per-instruction overhead.  The quarter blocks are contiguous,
            # so s_t is simply a pixel-major [Q, FI, NPROT] tensor.
            s_pm = s_t.rearrange("p (i n) -> p i n", n=NPROT)
            nchunk = 4 if w in (0, W - 1) else 2
            cw = FI // nchunk
            s_next = None
            for k in range(nchunk):
                i0 = k * cw
                if w == 1 and k == 1:
                    # chunk 1 reads quarters 2+3; absorb quarter 2's DMAHW
                    # tick so the first STT below carries only quarter 3's
                    probeb = mpool.tile([Q, 1], F32, tag="probeb", bufs=2)
                    i_pb = nc.vector.tensor_copy(
                        probeb, s_t[:, 2 * QB : 2 * QB + 1]
                    )
                    add_dep_helper(i_pb.ins, dve_prev.ins, sync=False)
                    dve_prev = i_pb
                # s16 = (mne * -1e4) + d, bf16 out, 8 protos per class block
                for c in range(C):
                    n0 = 8 * c
                    mne_b = (
                        mne_v[:, c, i0 : i0 + cw]
                        .unsqueeze(2)
                        .broadcast_to([Q, cw, 8])
                    )
                    i_stt = nc.vector.scalar_tensor_tensor(
                        s16_v[:, i0 : i0 + cw, n0 : n0 + 8],
                        mne_b,
                        -1.0e4,
                        s_pm[:, i0 : i0 + cw, n0 : n0 + 8],
                        mybir.AluOpType.mult,
                        mybir.AluOpType.add,
                    )
                    if c == 0:
                        add_dep_helper(i_stt.ins, dve_prev.ins, sync=False)
                    i_stt_last = i_stt

                act_prev = act_abs
                act_abs = None
                # ACT-side observer of the last STT of this chunk: the exp
                # below then sheds its DVE wait, and the prefetch DMA can
                # issue waitlessly right here.
                obs2 = mpool.tile([Q, 1], BF16, tag="obs2", bufs=8)
                i_obs2 = nc.scalar.copy(
                    obs2,
                    s16[:, (i0 + cw - 1) * NSLOT + 79 : (i0 + cw - 1) * NSLOT + 80],
                )
                if act_prev is not None:
                    add_dep_helper(i_obs2.ins, act_prev.ins, sync=False)
                act_prev = i_obs2
                if k == nchunk - 1 and w + 2 < W:
                    # All STTs of this window are done; prefetch window w+2
                    # into the freed buffer, half from ACT and half from
                    # GpSimd so the transfers spread across both engines'
                    # DMA queue sets.  1-elem copies absorb the four old
                    # quarter-transfers' DMAHW ticks (tile-granular release)
                    # plus the "STTs done" DVE tick on each issuing queue.
                    # All reads of the old tile are emitted BEFORE the new
                    # tile is allocated from the pool.
                    din_v = d_in.rearrange("(a q) m -> a q m", q=Q)
                    # DVE position-flag: emitted after the STTs, so a wait
                    # "DVE >= flag" implies all of window w's STTs are done.
                    # Reads only the probe tile, so s16 gains no new readers.
                    flag = mpool.tile([Q, 1], F32, tag="flag", bufs=2)
                    i_flag = nc.vector.tensor_copy(flag, probe)
                    add_dep_helper(i_flag.ins, i_stt_last.ins, sync=False)
                    # ACT-side DMAHW absorbers
                    prev = i_obs2
                    for k2 in range(4):
                        dmaobs = mpool.tile([Q, 1], F32, tag="dmaobs", bufs=8)
                        i_do = nc.scalar.copy(
                            dmaobs, s_t[:, k2 * QB : k2 * QB + 1]
                        )
                        add_dep_helper(i_do.ins, prev.ins, sync=False)
                        prev = i_do
                    dmaobs_last = dmaobs
                    # GpSimd-side absorbers (disjoint bytes from ACT's)
                    gflag = mpool.tile([Q, 1], F32, tag="gflag", bufs=2)
                    gprev = nc.gpsimd.tensor_copy(gflag, flag)
                    for k2 in range(4):
                        gdobs = mpool.tile([Q, 1], F32, tag="gdobs", bufs=8)
                        i_gdo = nc.gpsimd.tensor_copy(
                            gdobs, s_t[:, k2 * QB + 1 : k2 * QB + 2]
                        )
                        add_dep_helper(i_gdo.ins, gprev.ins, sync=False)
                        gprev = i_gdo
                    gdobs_last = gdobs
                    # cross-absorb: each issuing queue observes the OTHER
                    # queue's absorber chain (the tile-granular release makes
                    # both dmas depend on all accessors of the old tile)
                    xa = mpool.tile([Q, 1], F32, tag="xa", bufs=2)
                    i_xa = nc.scalar.copy(xa, gdobs_last)
                    add_dep_helper(i_xa.ins, prev.ins, sync=False)
                    xg = mpool.tile([Q, 1], F32, tag="xg", bufs=2)
                    i_xg = nc.gpsimd.tensor_copy(xg, dmaobs_last)
                    add_dep_helper(i_xg.ins, gprev.ins, sync=False)
                    # now recycle the buffer and issue both half-transfers
                    s_next = spool.tile(
                        [Q, NPROT * FI], F32, tag="s", name=f"s_t{w+2}"
                    )
                    s_tiles.append(s_next)
                    a0 = 4 * (w + 2)
                    i_dma = nc.scalar.dma_start(
                        out=s_next[:, 0 : 2 * QB].rearrange(
                            "p (x m) -> p x m", x=2
                        ),
                        in_=din_v[a0 : a0 + 2].transpose([1, 0, 2]),
                    )
                    add_dep_helper(i_dma.ins, i_xa.ins, sync=False)
                    act_prev = i_dma
                    i_gdma = nc.gpsimd.dma_start(
                        out=s_next[:, 2 * QB : 4 * QB].rearrange(
                            "p (x m) -> p x m", x=2
                        ),
                        in_=din_v[a0 + 2 : a0 + 4].transpose([1, 0, 2]),
                    )
                    add_dep_helper(i_gdma.ins, i_xg.ins, sync=False)

                # em = exp(s16), bf16
                i_exp = nc.scalar.activation(
                    em_v[:, i0 : i0 + cw, :],
                    s16_v[:, i0 : i0 + cw, :NPROT],
                    mybir.ActivationFunctionType.Exp,
                )
                if act_prev is not None:
                    add_dep_helper(i_exp.ins, act_prev.ins, sync=False)

                for i in range(i0, i0 + cw):
                    nc.tensor.matmul(
                        g_ps,
                        s16_v[:, i, :],
                        em_v[:, i, :],
                        start=first,
                        stop=(w == W - 1 and i == FI - 1),
                    )
                    first = False

            if w + 2 < W:
                # Read the accumulator right after this window's last matmul:
                # the copy waits exactly on "PE done with window w", putting
                # that tick into ACT's clock for window w+2's exp.
                dead4 = mpool.tile([1, 1], F32, tag="dead4", bufs=2)
                dead4_insts.append(nc.scalar.copy(dead4, g_ps[0:1, 0:1]))
                dead4_tiles.append(dead4)

        # Absorb the ACT-PSUM-read serialization into DVE so the final
        # PSUM->SBUF copy carries only the PE wait.
        deadf = mpool.tile([1, 1], F32, tag="deadf", bufs=1)
        i_deadf = nc.vector.tensor_copy(deadf, dead4_tiles[-1])
        g_sb = singles.tile([NSLOT, NPROT], F32)
        i_gcopy = nc.vector.tensor_copy(g_sb, g_ps)
        add_dep_helper(i_gcopy.ins, i_deadf.ins, sync=False)
        # Output DMA from ACT behind a g_sb observer, so the issue carries
        # at most the DMAHW semaphore-recycling wait.
        gobs = mpool.tile([1, 1], F32, tag="gobs", bufs=1)
        i_gobs = nc.scalar.copy(gobs, g_sb[0:1, 0:1])
        add_dep_helper(i_gobs.ins, i_gcopy.ins, sync=False)
        i_gdma = nc.scalar.dma_start(out=g_out[:, :], in_=g_sb)
        add_dep_helper(i_gdma.ins, i_gobs.ins, sync=False)

    # The kernel-tail drain aggregates every outstanding semaphore into one
    # instruction; the CTRL struct cannot hold that many waits.  Split it
    # into a chain of single-wait drains.
    import copy as _copy

    for fn in nc.m.functions:
        for blk in fn.blocks:
            insts = blk.instructions
            for idx, ins in enumerate(list(insts)):
                si = ins.sync_info
                if type(ins).__name__ == "InstDrain" and si and len(si.on_wait) > 1:
                    waits = list(si.on_wait)
                    si.on_wait = waits[-1:]
                    pos = insts.index(ins)
                    for k, wt in enumerate(waits[:-1]):
                        d2 = _copy.deepcopy(ins)
                        d2.name = f"{ins.name}-split{k}"
                        d2.sync_info = type(si)(on_wait=[wt], on_update=[])
                        insts.insert(pos + k, d2)
                    break

    return nc


def _get_nc():
    if "nc" not in _NC_CACHE:
        _NC_CACHE["nc"] = build_nc()
    return _NC_CACHE["nc"]


def run_device(dist8, labf8, trace=False):
    """dist8: [8, W*Q, NPROT*FI] f32 device layout; labf8: [8, P] f32 labels-1."""
    nc = _get_nc()
    cls = np.broadcast_to(np.arange(C, dtype=np.float32)[None, :], (Q, C))
    in_maps = []
    for b in range(B):
        labcls = np.concatenate([labf8[b].reshape(Q, 512), cls], axis=1)
        in_maps.append(
            {"dist": dist8[b], "labcls": np.ascontiguousarray(labcls)}
        )
    return run_bass_kernel_spmd(nc, in_maps, list(range(B)), trace=trace)


def kernel(
    prototype_distances,
    target_labels,
    proto_class,
    pair_i,
    pair_j,
    pair_cls,
    _trace=False,
    _results_out=None,
):
    dist = np.asarray(prototype_distances, dtype=np.float32).reshape(B, NPROT, P)
    labels = np.asarray(target_labels).reshape(B, P).astype(np.int64)
    proto_class = np.asarray(proto_class, dtype=np.int64)
    pair_i = np.asarray(pair_i, dtype=np.int64)
    pair_j = np.asarray(pair_j, dtype=np.int64)
    pair_cls = np.asarray(pair_cls, dtype=np.int64)

    # Permute prototypes to class-major layout: slot n -> class n // 8.
    perm = np.empty(NPROT, dtype=np.int64)
    for c in range(C):
        protos = np.nonzero(proto_class == c)[0]
        assert len(protos) == 8, "expect 8 prototypes per class"
        perm[8 * c : 8 * c + 8] = protos
    inv = np.empty(NPROT, dtype=np.int64)
    inv[perm] = np.arange(NPROT)

    # Device layout [w, quarter, q, n, i]: pixel p = 512*q + 128*w + 32*k + i,
    # protos class-major.  One transpose+copy host-side buys fully
    # contiguous quarter-window device DMAs.
    QW_ = FI // 4
    dist_v = dist[:, perm, :].reshape(B, NPROT, Q, W, 4, QW_)
    dist_p = np.ascontiguousarray(dist_v.transpose(0, 3, 4, 2, 5, 1)).reshape(
        B, W * 4 * Q, NPROT * QW_
    )
    labf = np.ascontiguousarray((labels - 1).astype(np.float32))

    br = run_device(dist_p, labf, trace=_trace)
    if _results_out is not None:
        _results_out.append(br)

    total_vals = np.float64(0.0)
    total_valid = 0
    for b in range(B):
        out = br.results[b]["g"]  # [81, 80]; out[j, a] = G[a, j], out[80, a] = Z_a
        Z = out[NPROT].astype(np.float64)
        Gt = out[:NPROT].astype(np.float64)  # Gt[j, a] = sum_p em_a * s_j
        with np.errstate(divide="ignore", invalid="ignore"):
            A = np.where(Z[None, :] != 0.0, Gt / Z[None, :], 0.0)  # A[j, a] = E_a[d_j]
        lb = labels[b] - 1
        cnt = np.bincount(lb[lb >= 0], minlength=C)
        ii = inv[pair_i]
        jj = inv[pair_j]
        # A[x, a] = expectation of d_x under softmax of proto a
        kld = 0.5 * (A[jj, jj] - A[jj, ii] + A[ii, ii] - A[ii, jj])
        valid = cnt[pair_cls] >= 2
        total_vals += np.exp(-kld[valid]).sum()
        total_valid += int(valid.sum())

    if total_valid > 0:
        res = np.float32(total_vals / max(total_valid, 1))
    else:
        res = np.float32(0.0)
    return res


if __name__ == "__main__":
    rng = np.random.default_rng(0)
    d = rng.standard_normal((B, NPROT, 256, 256), dtype=np.float32)
    l = rng.integers(0, 11, (B, 256, 256))
    pc = (np.arange(NPROT) % 40) // 4
    pairs = []
    for s in range(2):
        for c in range(C):
            base = s * 40 + c * 4
            for a in range(4):
                for b2 in range(a + 1, 4):
                    pairs.append((base + a, base + b2, c))
    pairs = np.asarray(pairs, np.int32)
    print(kernel(d, l, pc, pairs[:, 0], pairs[:, 1], pairs[:, 2]))


# revision 47
# speedup vs baseline: 1.2049x; 1.2049x over previous
"""Trainium2 Bass kernel for nn_KLDLoss_18769007083961.

Math reformulation (validated vs reference, rel err ~3e-5 with bf16):
  For each image b, prototype a with class c(a), define over pixels p:
    s_a[p]  = d_a[p] + (label[p] != c(a)) * (-1e4)      # masked-biased distance
    em_a[p] = exp(s_a[p])                               # exactly 0 off-class (underflow)
    Z_a     = sum_p em_a[p]
    G[a,j]  = sum_p em_a[p] * s_j[p]   (j in same group => same class mask)
    A[a,j]  = G[a,j] / Z_a
  Symmetric KL for a same-group pair (i,j) (log-partition terms cancel):
    kld = 0.5 * (A[j,j] - A[j,i] + A[i,i] - A[i,j])
  loss = mean over valid pairs (class count >= 2) of exp(-kld).

Only same-class G entries are consumed, and em is exactly zero off-class,
so the bf16 rounding of the -1e4 bias in s never reaches the result: the
biased s tile can be written once in bf16 and feed both the exp (ACT) and
the matmul lhsT (PE).  bf16 matmul runs at 1 cycle/row vs fp32's 4.

Device kernel (one image per NeuronCore, 8 cores):
  Layout: pixel p = 512*q + 128*w + i  (q = SBUF partition, w = window, i = inner).
  Protos are permuted host-side to class-major order (slot = 8c + 4s + m) so the
  class bias applies to 8 contiguous protos per DVE op.
  Per window: DMA dist -> s_t[128, 80*128] f32; DVE builds (label != c) and
  writes s16[128, 81*128] bf16 = d + mne*-1e4 (slot 80 memset to 1.0);
  ACT computes em = exp(s16) bf16; 128 matmuls (lhsT = s16-slice [128,81],
  rhs = em-slice [128,80]) accumulate out[j,a] = G[a,j], row 80 = Z, in
  PSUM [81,80] f32.  The last window runs in 4 column chunks to shorten the
  pipeline tail after the final DMA.  Host does the tiny 120-pair combine.
"""

import sys
from contextlib import ExitStack

import ml_dtypes
import numpy as np

sys.path.insert(0, "/opt/trn_rl_repo")

import concourse.bass as bass
import concourse.tile as tile
from concourse import mybir
from concourse.bass_utils import run_bass_kernel_spmd
from concourse.tile import add_dep_helper

B = 8
C = 10
NPROT = 80
NSLOT = NPROT + 1  # 80 protos + ones column for Z
P = 65536
Q = 128          # partitions = coarse pixel blocks of 512
W = 4            # windows per image
FI = 128         # inner pixels per window per partition
F32 = mybir.dt.float32
BF16 = mybir.dt.bfloat16

_NC_CACHE = {}


def build_nc():
    nc = bass.Bass()
    # dist pre-transposed host-side to [w, quarter, q, n, i]: every quarter-
    # window DMA is a plain contiguous copy, so the first STT chunk starts
    # after ~1/16 of the image has landed.
    d_in = nc.dram_tensor(
        "dist", [W * 4 * Q, NPROT * (FI // 4)], F32, kind="ExternalInput"
    )
    # labels [q, 512] packed with the 10 class constants -> cols 512..521
    lab_in = nc.dram_tensor("labcls", [Q, 512 + C], BF16, kind="ExternalInput")
    g_out = nc.dram_tensor("g", [NSLOT, NPROT], F32, kind="ExternalOutput")

    with ExitStack() as ctx:
        tc = ctx.enter_context(tile.TileContext(nc))
        singles = ctx.enter_context(tc.tile_pool(name="singles", bufs=1))
        spool = ctx.enter_context(tc.tile_pool(name="spool", bufs=3))
        s16pool = ctx.enter_context(tc.tile_pool(name="s16pool", bufs=2))
        empool = ctx.enter_context(tc.tile_pool(name="empool", bufs=2))
        mpool = ctx.enter_context(tc.tile_pool(name="mpool", bufs=2))
        psum = ctx.enter_context(tc.tile_pool(name="psum", bufs=1, space="PSUM"))

        labels_t = singles.tile([Q, 512 + C], BF16)
        nc.sync.dma_start(out=labels_t, in_=lab_in[:, :])
        cls_t = labels_t[:, 512 : 512 + C]

        g_ps = psum.tile([NSLOT, NPROT], F32)

        QW_ = FI // 4  # pixels per quarter-window
        QB = NPROT * QW_  # sbuf columns per quarter block
        s_tiles = []
        for w in range(3):
            s_w = spool.tile([Q, NPROT * FI], F32, tag="s", name=f"s_t{w}")
            for k in range(4):
                nc.sync.dma_start(
                    out=s_w[:, k * QB : (k + 1) * QB],
                    in_=d_in[(4 * w + k) * Q : (4 * w + k + 1) * Q, :],
                )
            s_tiles.append(s_w)


        # Engines have a single sync-wait slot per instruction.  Every
        # cross-engine dependency is therefore carried by a dedicated
        # 1-element absorber op, pinned ahead of its consumer with no-sync
        # dep edges so the scheduler keeps the elision-enabling order.
        first = True
        em_tiles = []
        dead4_tiles = []
        dead4_insts = []
        for w in range(W):
            s_t = s_tiles[w]

            # mne[p, c, i] = (labels != c) as 1.0/0.0, bf16
            mne = mpool.tile([Q, C * FI], BF16, tag="mne")
            mne_v = mne.rearrange("p (c i) -> p c i", c=C)
            lab_w = labels_t[:, w * FI : (w + 1) * FI]
            nc.vector.tensor_tensor(
                mne_v,
                lab_w.unsqueeze(1).broadcast_to([Q, C, FI]),
                cls_t.unsqueeze(2).broadcast_to([Q, C, FI]),
                mybir.AluOpType.not_equal,
            )

            # DVE absorber chain: (1) dist-DMA completion for this window
            probe = mpool.tile([Q, 1], F32, tag="probe", bufs=4)
            i_probe = nc.vector.tensor_copy(probe, s_t[:, 0:1])
            dve_prev = i_probe
            if w >= 2:
                # (2) ACT finished exp(w-2) (read byte from its LAST chunk),
                # which read the s16 buffer the STTs below recycle
                probe2 = mpool.tile([Q, 1], BF16, tag="probe2", bufs=4)
                i_probe2 = nc.vector.tensor_copy(
                    probe2,
                    em_tiles[w - 2][:, (FI - 1) * NPROT : (FI - 1) * NPROT + 1],
                )
                add_dep_helper(i_probe2.ins, dve_prev.ins, sync=False)
                dve_prev = i_probe2

            s16 = s16pool.tile([Q, NSLOT * FI], BF16, tag="s16")
            s16_v = s16.rearrange("p (i n) -> p i n", n=NSLOT)
            em = empool.tile([Q, NPROT * FI], BF16, tag="em")
            em_v = em.rearrange("p (i n) -> p i n", n=NPROT)
            em_tiles.append(em)

            # ones column (slot 80) -> Z row of the gram.  For w >= 2 its
            # bytes were read by every LDW of window w-2, so this memset
            # carries exactly the "PE done with window w-2" wait that the
            # STTs would otherwise each need.
            i_memset = nc.vector.memset(s16_v[:, :, NPROT], 1.0)
            add_dep_helper(i_memset.ins, dve_prev.ins, sync=False)
            dve_prev = i_memset

            # ACT absorber: reading an old-em byte absorbs the same-engine
            # WAW tick (exp(w) overwrites exp(w-2)'s output); the PE tick was
            # absorbed by dead_act at the end of window w-2.
            act_abs = None
            if w >= 2:
                # Read a byte exp(w-2)'s LAST chunk wrote: the single wait
                # "ACT >= exp(w-2, k3)" dominates every chunk's WAW below.
                dead3 = mpool.tile([Q, 1], BF16, tag="dead3", bufs=2)
                act_abs = nc.scalar.copy(
                    dead3,
                    em_tiles[w - 2][
                        :, (FI - 1) * NPROT + 1 : (FI - 1) * NPROT + 2
                    ],
                )
                add_dep_helper(act_abs.ins, dead4_insts[w - 2].ins, sync=False)

            # Chunked pipeline: fine chunks on the first/last window for a
            # fast start/short tail, coarser in the middle to cut DVE
            # per-instruction overhead.  The quarter blocks are contiguous,
            # so s_t is simply a pixel-major [Q, FI, NPROT] tensor.
            s_pm = s_t.rearrange("p (i n) -> p i n", n=NPROT)
            nchunk = 4 if w in (0, W - 1) else 2
            cw = FI // nchunk
            s_next = None
            for k in range(nchunk):
                i0 = k * cw
                if w in (1, 2) and k == 1:
                    # chunk 1 reads quarters 2+3; absorb quarter 2's DMAHW
                    # tick so the first STT below carries only quarter 3's
                    probeb = mpool.tile([Q, 1], F32, tag="probeb", bufs=4)
                    i_pb = nc.vector.tensor_copy(
                        probeb, s_t[:, 2 * QB : 2 * QB + 1]
                    )
                    add_dep_helper(i_pb.ins, dve_prev.ins, sync=False)
                    dve_prev = i_pb
                # s16 = (mne * -1e4) + d, bf16 out, 8 protos per class block
                for c in range(C):
                    n0 = 8 * c
                    mne_b = (
                        mne_v[:, c, i0 : i0 + cw]
                        .unsqueeze(2)
                        .broadcast_to([Q, cw, 8])
                    )
                    i_stt = nc.vector.scalar_tensor_tensor(
                        s16_v[:, i0 : i0 + cw, n0 : n0 + 8],
                        mne_b,
                        -1.0e4,
                        s_pm[:, i0 : i0 + cw, n0 : n0 + 8],
                        mybir.AluOpType.mult,
                        mybir.AluOpType.add,
                    )
                    if c == 0:
                        add_dep_helper(i_stt.ins, dve_prev.ins, sync=False)
                    i_stt_last = i_stt

                act_prev = act_abs
                act_abs = None
                # ACT-side observer of the last STT of this chunk: the exp
                # below then sheds its DVE wait, and the prefetch DMA can
                # issue waitlessly right here.
                obs2 = mpool.tile([Q, 1], BF16, tag="obs2", bufs=8)
                i_obs2 = nc.scalar.copy(
                    obs2,
                    s16[:, (i0 + cw - 1) * NSLOT + 79 : (i0 + cw - 1) * NSLOT + 80],
                )
                if act_prev is not None:
                    add_dep_helper(i_obs2.ins, act_prev.ins, sync=False)
                act_prev = i_obs2
                if k == nchunk - 1 and w + 3 < W:
                    # All of window 0's STTs are done; prefetch window 3 into
                    # the freed buffer from ACT.  The 1-elem copies absorb
                    # the four old quarter-transfers' DMAHW ticks
                    # (tile-granular release); the DVE tick came via obs2.
                    din_v = d_in.rearrange("(a q) m -> a q m", q=Q)
                    prev = i_obs2
                    for k2 in range(4):
                        dmaobs = mpool.tile([Q, 1], F32, tag="dmaobs", bufs=4)
                        i_do = nc.scalar.copy(
                            dmaobs, s_t[:, k2 * QB : k2 * QB + 1]
                        )
                        add_dep_helper(i_do.ins, prev.ins, sync=False)
                        prev = i_do
                    s_next = spool.tile(
                        [Q, NPROT * FI], F32, tag="s", name=f"s_t{w+3}"
                    )
                    s_tiles.append(s_next)
                    a0 = 4 * (w + 3)
                    for h2 in range(2):
                        i_dma = nc.scalar.dma_start(
                            out=s_next[
                                :, 2 * h2 * QB : 2 * (h2 + 1) * QB
                            ].rearrange("p (x m) -> p x m", x=2),
                            in_=din_v[a0 + 2 * h2 : a0 + 2 * h2 + 2].transpose(
                                [1, 0, 2]
                            ),
                        )
                        add_dep_helper(i_dma.ins, prev.ins, sync=False)
                        prev = i_dma
                    act_prev = prev

                # em = exp(s16), bf16
                i_exp = nc.scalar.activation(
                    em_v[:, i0 : i0 + cw, :],
                    s16_v[:, i0 : i0 + cw, :NPROT],
                    mybir.ActivationFunctionType.Exp,
                )
                if act_prev is not None:
                    add_dep_helper(i_exp.ins, act_prev.ins, sync=False)

                for i in range(i0, i0 + cw):
                    nc.tensor.matmul(
                        g_ps,
                        s16_v[:, i, :],
                        em_v[:, i, :],
                        start=first,
                        stop=(w == W - 1 and i == FI - 1),
                    )
                    first = False

            if w + 2 < W:
                # Read the accumulator right after this window's last matmul:
                # the copy waits exactly on "PE done with window w", putting
                # that tick into ACT's clock for window w+2's exp.
                dead4 = mpool.tile([1, 1], F32, tag="dead4", bufs=2)
                dead4_insts.append(nc.scalar.copy(dead4, g_ps[0:1, 0:1]))
                dead4_tiles.append(dead4)

        # Absorb the ACT-PSUM-read serialization into DVE so the final
        # PSUM->SBUF copy carries only the PE wait.
        deadf = mpool.tile([1, 1], F32, tag="deadf", bufs=1)
        i_deadf = nc.vector.tensor_copy(deadf, dead4_tiles[-1])
        g_sb = singles.tile([NSLOT, NPROT], F32)
        i_gcopy = nc.vector.tensor_copy(g_sb, g_ps)
        add_dep_helper(i_gcopy.ins, i_deadf.ins, sync=False)
        # Output DMA from ACT behind a g_sb observer, so the issue carries
        # at most the DMAHW semaphore-recycling wait.
        gobs = mpool.tile([1, 1], F32, tag="gobs", bufs=1)
        i_gobs = nc.scalar.copy(gobs, g_sb[0:1, 0:1])
        add_dep_helper(i_gobs.ins, i_gcopy.ins, sync=False)
        i_gdma = nc.scalar.dma_start(out=g_out[:, :], in_=g_sb)
        add_dep_helper(i_gdma.ins, i_gobs.ins, sync=False)

    # The kernel-tail drain aggregates every outstanding semaphore into one
    # instruction; the CTRL struct cannot hold that many waits.  Split it
    # into a chain of single-wait drains.
    import copy as _copy

    for fn in nc.m.functions:
        for blk in fn.blocks:
            insts = blk.instructions
            for idx, ins in enumerate(list(insts)):
                si = ins.sync_info
                if type(ins).__name__ == "InstDrain" and si and len(si.on_wait) > 1:
                    waits = list(si.on_wait)
                    si.on_wait = waits[-1:]
                    pos = insts.index(ins)
                    for k, wt in enumerate(waits[:-1]):
                        d2 = _copy.deepcopy(ins)
                        d2.name = f"{ins.name}-split{k}"
                        d2.sync_info = type(si)(on_wait=[wt], on_update=[])
                        insts.insert(pos + k, d2)
                    break

    return nc


def _get_nc():
    if "nc" not in _NC_CACHE:
        _NC_CACHE["nc"] = build_nc()
    return _NC_CACHE["nc"]


def run_device(dist8, labf8, trace=False):
    """dist8: [8, W*Q, NPROT*FI] f32 device layout; labf8: [8, P] f32 labels-1."""
    nc = _get_nc()
    cls = np.broadcast_to(np.arange(C, dtype=np.float32)[None, :], (Q, C))
    in_maps = []
    for b in range(B):
        labcls = np.concatenate([labf8[b].reshape(Q, 512), cls], axis=1)
        in_maps.append(
            {
                "dist": dist8[b],
                "labcls": np.ascontiguousarray(labcls).astype(ml_dtypes.bfloat16),
            }
        )
    return run_bass_kernel_spmd(nc, in_maps, list(range(B)), trace=trace)


def kernel(
    prototype_distances,
    target_labels,
    proto_class,
    pair_i,
    pair_j,
    pair_cls,
    _trace=False,
    _results_out=None,
):
    dist = np.asarray(prototype_distances, dtype=np.float32).reshape(B, NPROT, P)
    labels = np.asarray(target_labels).reshape(B, P).astype(np.int64)
    proto_class = np.asarray(proto_class, dtype=np.int64)
    pair_i = np.asarray(pair_i, dtype=np.int64)
    pair_j = np.asarray(pair_j, dtype=np.int64)
    pair_cls = np.asarray(pair_cls, dtype=np.int64)

    # Permute prototypes to class-major layout: slot n -> class n // 8.
    perm = np.empty(NPROT, dtype=np.int64)
    for c in range(C):
        protos = np.nonzero(proto_class == c)[0]
        assert len(protos) == 8, "expect 8 prototypes per class"
        perm[8 * c : 8 * c + 8] = protos
    inv = np.empty(NPROT, dtype=np.int64)
    inv[perm] = np.arange(NPROT)

    # Device layout [w, quarter, q, n, i]: pixel p = 512*q + 128*w + 32*k + i,
    # protos class-major.  One transpose+copy host-side buys fully
    # contiguous quarter-window device DMAs.
    QW_ = FI // 4
    dist_v = dist[:, perm, :].reshape(B, NPROT, Q, W, 4, QW_)
    dist_p = np.ascontiguousarray(dist_v.transpose(0, 3, 4, 2, 5, 1)).reshape(
        B, W * 4 * Q, NPROT * QW_
    )
    labf = np.ascontiguousarray((labels - 1).astype(np.float32))

    br = run_device(dist_p, labf, trace=_trace)
    if _results_out is not None:
        _results_out.append(br)

    total_vals = np.float64(0.0)
    total_valid = 0
    for b in range(B):
        out = br.results[b]["g"]  # [81, 80]; out[j, a] = G[a, j], out[80, a] = Z_a
        Z = out[NPROT].astype(np.float64)
        Gt = out[:NPROT].astype(np.float64)  # Gt[j, a] = sum_p em_a * s_j
        with np.errstate(divide="ignore", invalid="ignore"):
            A = np.where(Z[None, :] != 0.0, Gt / Z[None, :], 0.0)  # A[j, a] = E_a[d_j]
        lb = labels[b] - 1
        cnt = np.bincount(lb[lb >= 0], minlength=C)
        ii = inv[pair_i]
        jj = inv[pair_j]
        # A[x, a] = expectation of d_x under softmax of proto a
        kld = 0.5 * (A[jj, jj] - A[jj, ii] + A[ii, ii] - A[ii, jj])
        valid = cnt[pair_cls] >= 2
        total_vals += np.exp(-kld[valid]).sum()
        total_valid += int(valid.sum())

    if total_valid > 0:
        res = np.float32(total_vals / max(total_valid, 1))
    else:
        res = np.float32(0.0)
    return res


if __name__ == "__main__":
    rng = np.random.default_rng(0)
    d = rng.standard_normal((B, NPROT, 256, 256), dtype=np.float32)
    l = rng.integers(0, 11, (B, 256, 256))
    pc = (np.arange(NPROT) % 40) // 4
    pairs = []
    for s in range(2):
        for c in range(C):
            base = s * 40 + c * 4
            for a in range(4):
                for b2 in range(a + 1, 4):
                    pairs.append((base + a, base + b2, c))
    pairs = np.asarray(pairs, np.int32)
    print(kernel(d, l, pc, pairs[:, 0], pairs[:, 1], pairs[:, 2]))


# revision 51
# speedup vs baseline: 1.2083x; 1.0028x over previous
"""Trainium2 Bass kernel for nn_KLDLoss_18769007083961.

Math reformulation (validated vs reference, rel err ~3e-5 with bf16):
  For each image b, prototype a with class c(a), define over pixels p:
    s_a[p]  = d_a[p] + (label[p] != c(a)) * (-1e4)      # masked-biased distance
    em_a[p] = exp(s_a[p])                               # exactly 0 off-class (underflow)
    Z_a     = sum_p em_a[p]
    G[a,j]  = sum_p em_a[p] * s_j[p]   (j in same group => same class mask)
    A[a,j]  = G[a,j] / Z_a
  Symmetric KL for a same-group pair (i,j) (log-partition terms cancel):
    kld = 0.5 * (A[j,j] - A[j,i] + A[i,i] - A[i,j])
  loss = mean over valid pairs (class count >= 2) of exp(-kld).

Only same-class G entries are consumed, and em is exactly zero off-class,
so the bf16 rounding of the -1e4 bias in s never reaches the result: the
biased s tile can be written once in bf16 and feed both the exp (ACT) and
the matmul lhsT (PE).  bf16 matmul runs at 1 cycle/row vs fp32's 4.

Device kernel (one image per NeuronCore, 8 cores):
  Layout: pixel p = 512*q + 128*w + i  (q = SBUF partition, w = window, i = inner).
  Protos are permuted host-side to class-major order (slot = 8c + 4s + m) so the
  class bias applies to 8 contiguous protos per DVE op.
  Per window: DMA dist -> s_t[128, 80*128] f32; DVE builds (label != c) and
  writes s16[128, 81*128] bf16 = d + mne*-1e4 (slot 80 memset to 1.0);
  ACT computes em = exp(s16) bf16; 128 matmuls (lhsT = s16-slice [128,81],
  rhs = em-slice [128,80]) accumulate out[j,a] = G[a,j], row 80 = Z, in
  PSUM [81,80] f32.  The last window runs in 4 column chunks to shorten the
  pipeline tail after the final DMA.  Host does the tiny 120-pair combine.
"""

import sys
from contextlib import ExitStack

import ml_dtypes
import numpy as np

sys.path.insert(0, "/opt/trn_rl_repo")

import concourse.bass as bass
import concourse.tile as tile
from concourse import mybir
from concourse.bass_utils import run_bass_kernel_spmd
from concourse.tile import add_dep_helper

B = 8
C = 10
NPROT = 80
NSLOT = NPROT + 1  # 80 protos + ones column for Z
P = 65536
Q = 128          # partitions = coarse pixel blocks of 512
W = 4            # windows per image
FI = 128         # inner pixels per window per partition
F32 = mybir.dt.float32
BF16 = mybir.dt.bfloat16

_NC_CACHE = {}


def build_nc():
    nc = bass.Bass()
    # dist pre-transposed host-side to [w, quarter, q, n, i]: every quarter-
    # window DMA is a plain contiguous copy, so the first STT chunk starts
    # after ~1/16 of the image has landed.
    d_in = nc.dram_tensor(
        "dist", [W * 4 * Q, NPROT * (FI // 4)], F32, kind="ExternalInput"
    )
    # labels [q, 512] packed with the 10 class constants -> cols 512..521
    lab_in = nc.dram_tensor("labcls", [Q, 512 + C], BF16, kind="ExternalInput")
    g_out = nc.dram_tensor("g", [NSLOT, NPROT], F32, kind="ExternalOutput")

    with ExitStack() as ctx:
        tc = ctx.enter_context(tile.TileContext(nc))
        singles = ctx.enter_context(tc.tile_pool(name="singles", bufs=1))
        spool = ctx.enter_context(tc.tile_pool(name="spool", bufs=3))
        s16pool = ctx.enter_context(tc.tile_pool(name="s16pool", bufs=2))
        empool = ctx.enter_context(tc.tile_pool(name="empool", bufs=2))
        mpool = ctx.enter_context(tc.tile_pool(name="mpool", bufs=2))
        psum = ctx.enter_context(tc.tile_pool(name="psum", bufs=1, space="PSUM"))

        labels_t = singles.tile([Q, 512 + C], BF16)
        nc.sync.dma_start(out=labels_t, in_=lab_in[:, :])
        cls_t = labels_t[:, 512 : 512 + C]

        g_ps = psum.tile([NSLOT, NPROT], F32)

        QW_ = FI // 4  # pixels per quarter-window
        QB = NPROT * QW_  # sbuf columns per quarter block
        s_tiles = []
        for w in range(3):
            s_w = spool.tile([Q, NPROT * FI], F32, tag="s", name=f"s_t{w}")
            for k in range(4):
                nc.sync.dma_start(
                    out=s_w[:, k * QB : (k + 1) * QB],
                    in_=d_in[(4 * w + k) * Q : (4 * w + k + 1) * Q, :],
                )
            s_tiles.append(s_w)


        # Engines have a single sync-wait slot per instruction.  Every
        # cross-engine dependency is therefore carried by a dedicated
        # 1-element absorber op, pinned ahead of its consumer with no-sync
        # dep edges so the scheduler keeps the elision-enabling order.
        first = True
        em_tiles = []
        dead4_tiles = []
        dead4_insts = []
        for w in range(W):
            s_t = s_tiles[w]

            # mne[p, c, i] = (labels != c) as 1.0/0.0, bf16
            mne = mpool.tile([Q, C * FI], BF16, tag="mne")
            mne_v = mne.rearrange("p (c i) -> p c i", c=C)
            lab_w = labels_t[:, w * FI : (w + 1) * FI]
            nc.vector.tensor_tensor(
                mne_v,
                lab_w.unsqueeze(1).broadcast_to([Q, C, FI]),
                cls_t.unsqueeze(2).broadcast_to([Q, C, FI]),
                mybir.AluOpType.not_equal,
            )

            # DVE absorber chain: (1) dist-DMA completion for this window
            probe = mpool.tile([Q, 1], F32, tag="probe", bufs=4)
            i_probe = nc.vector.tensor_copy(probe, s_t[:, 0:1])
            dve_prev = i_probe
            if w >= 2:
                # (2) ACT finished exp(w-2) (read byte from its LAST chunk),
                # which read the s16 buffer the STTs below recycle
                probe2 = mpool.tile([Q, 1], BF16, tag="probe2", bufs=4)
                i_probe2 = nc.vector.tensor_copy(
                    probe2,
                    em_tiles[w - 2][:, (FI - 1) * NPROT : (FI - 1) * NPROT + 1],
                )
                add_dep_helper(i_probe2.ins, dve_prev.ins, sync=False)
                dve_prev = i_probe2

            s16 = s16pool.tile([Q, NSLOT * FI], BF16, tag="s16")
            s16_v = s16.rearrange("p (i n) -> p i n", n=NSLOT)
            em = empool.tile([Q, NPROT * FI], BF16, tag="em")
            em_v = em.rearrange("p (i n) -> p i n", n=NPROT)
            em_tiles.append(em)

            # ones column (slot 80) -> Z row of the gram.  For w >= 2 its
            # bytes were read by every LDW of window w-2, so this memset
            # carries exactly the "PE done with window w-2" wait that the
            # STTs would otherwise each need.
            i_memset = nc.vector.memset(s16_v[:, :, NPROT], 1.0)
            add_dep_helper(i_memset.ins, dve_prev.ins, sync=False)
            dve_prev = i_memset

            # ACT absorber: reading an old-em byte absorbs the same-engine
            # WAW tick (exp(w) overwrites exp(w-2)'s output); the PE tick was
            # absorbed by dead_act at the end of window w-2.
            act_abs = None
            if w >= 2:
                # Read a byte exp(w-2)'s LAST chunk wrote: the single wait
                # "ACT >= exp(w-2, k3)" dominates every chunk's WAW below.
                dead3 = mpool.tile([Q, 1], BF16, tag="dead3", bufs=2)
                act_abs = nc.scalar.copy(
                    dead3,
                    em_tiles[w - 2][
                        :, (FI - 1) * NPROT + 1 : (FI - 1) * NPROT + 2
                    ],
                )
                add_dep_helper(act_abs.ins, dead4_insts[w - 2].ins, sync=False)

            # Chunked pipeline: fine chunks on the first/last window for a
            # fast start/short tail, coarser in the middle to cut DVE
            # per-instruction overhead.  The quarter blocks are contiguous,
            # so s_t is simply a pixel-major [Q, FI, NPROT] tensor.
            s_pm = s_t.rearrange("p (i n) -> p i n", n=NPROT)
            nchunk = 4 if w in (0, W - 1) else 2
            cw = FI // nchunk
            s_next = None
            for k in range(nchunk):
                i0 = k * cw
                if w in (1, 2) and k == 1:
                    # chunk 1 reads quarters 2+3; absorb quarter 2's DMAHW
                    # tick so the first STT below carries only quarter 3's
                    probeb = mpool.tile([Q, 1], F32, tag="probeb", bufs=4)
                    i_pb = nc.vector.tensor_copy(
                        probeb, s_t[:, 2 * QB : 2 * QB + 1]
                    )
                    add_dep_helper(i_pb.ins, dve_prev.ins, sync=False)
                    dve_prev = i_pb
                # s16 = (mne * -1e4) + d, bf16 out, 8 protos per class block
                for c in range(C):
                    n0 = 8 * c
                    mne_b = (
                        mne_v[:, c, i0 : i0 + cw]
                        .unsqueeze(2)
                        .broadcast_to([Q, cw, 8])
                    )
                    i_stt = nc.vector.scalar_tensor_tensor(
                        s16_v[:, i0 : i0 + cw, n0 : n0 + 8],
                        mne_b,
                        -1.0e4,
                        s_pm[:, i0 : i0 + cw, n0 : n0 + 8],
                        mybir.AluOpType.mult,
                        mybir.AluOpType.add,
                    )
                    if c == 0:
                        add_dep_helper(i_stt.ins, dve_prev.ins, sync=False)
                    i_stt_last = i_stt

                act_prev = act_abs
                act_abs = None
                # ACT-side observer of the last STT of this chunk: the exp
                # below then sheds its DVE wait, and the prefetch DMA can
                # issue waitlessly right here.
                obs2 = mpool.tile([Q, 1], BF16, tag="obs2", bufs=8)
                i_obs2 = nc.scalar.copy(
                    obs2,
                    s16[:, (i0 + cw - 1) * NSLOT + 79 : (i0 + cw - 1) * NSLOT + 80],
                )
                if act_prev is not None:
                    add_dep_helper(i_obs2.ins, act_prev.ins, sync=False)
                act_prev = i_obs2
                if k == nchunk - 1 and w + 3 < W:
                    # All of window 0's STTs are done; prefetch window 3 into
                    # the freed buffer from ACT.  The 1-elem copies absorb
                    # the four old quarter-transfers' DMAHW ticks
                    # (tile-granular release); the DVE tick came via obs2.
                    din_v = d_in.rearrange("(a q) m -> a q m", q=Q)
                    prev = i_obs2
                    for k2 in range(4):
                        dmaobs = mpool.tile([Q, 1], F32, tag="dmaobs", bufs=4)
                        i_do = nc.scalar.copy(
                            dmaobs, s_t[:, k2 * QB : k2 * QB + 1]
                        )
                        add_dep_helper(i_do.ins, prev.ins, sync=False)
                        prev = i_do
                    s_next = spool.tile(
                        [Q, NPROT * FI], F32, tag="s", name=f"s_t{w+3}"
                    )
                    s_tiles.append(s_next)
                    a0 = 4 * (w + 3)
                    for h2 in range(2):
                        i_dma = nc.scalar.dma_start(
                            out=s_next[
                                :, 2 * h2 * QB : 2 * (h2 + 1) * QB
                            ].rearrange("p (x m) -> p x m", x=2),
                            in_=din_v[a0 + 2 * h2 : a0 + 2 * h2 + 2].transpose(
                                [1, 0, 2]
                            ),
                        )
                        add_dep_helper(i_dma.ins, prev.ins, sync=False)
                        prev = i_dma
                    act_prev = prev

                # em = exp(s16), bf16
                i_exp = nc.scalar.activation(
                    em_v[:, i0 : i0 + cw, :],
                    s16_v[:, i0 : i0 + cw, :NPROT],
                    mybir.ActivationFunctionType.Exp,
                )
                if act_prev is not None:
                    add_dep_helper(i_exp.ins, act_prev.ins, sync=False)

                for i in range(i0, i0 + cw):
                    nc.tensor.matmul(
                        g_ps,
                        s16_v[:, i, :],
                        em_v[:, i, :],
                        start=first,
                        stop=(w == W - 1 and i == FI - 1),
                    )
                    first = False

            if w + 2 < W:
                # Read the accumulator right after this window's last matmul:
                # the copy waits exactly on "PE done with window w", putting
                # that tick into ACT's clock for window w+2's exp.
                dead4 = mpool.tile([1, 1], F32, tag="dead4", bufs=2)
                dead4_insts.append(nc.scalar.copy(dead4, g_ps[0:1, 0:1]))
                dead4_tiles.append(dead4)

        # Absorb the ACT-PSUM-read serialization into DVE so the final
        # PSUM->SBUF copy carries only the PE wait.
        deadf = mpool.tile([1, 1], F32, tag="deadf", bufs=1)
        i_deadf = nc.vector.tensor_copy(deadf, dead4_tiles[-1])
        g_sb = singles.tile([NSLOT, NPROT], F32)
        i_gcopy = nc.vector.tensor_copy(g_sb, g_ps)
        add_dep_helper(i_gcopy.ins, i_deadf.ins, sync=False)
        # Output DMA from ACT behind a g_sb observer, so the issue carries
        # at most the DMAHW semaphore-recycling wait.
        gobs = mpool.tile([1, 1], F32, tag="gobs", bufs=1)
        i_gobs = nc.scalar.copy(gobs, g_sb[0:1, 0:1])
        add_dep_helper(i_gobs.ins, i_gcopy.ins, sync=False)
        i_gdma = nc.scalar.dma_start(out=g_out[:, :], in_=g_sb)
        add_dep_helper(i_gdma.ins, i_gobs.ins, sync=False)

    # The kernel-tail drain aggregates every outstanding semaphore into one
    # instruction; the CTRL struct cannot hold that many waits.  Split it
    # into a chain of single-wait drains.
    import copy as _copy

    for fn in nc.m.functions:
        for blk in fn.blocks:
            insts = blk.instructions
            for idx, ins in enumerate(list(insts)):
                si = ins.sync_info
                if type(ins).__name__ == "InstDrain" and si and len(si.on_wait) > 1:
                    waits = list(si.on_wait)
                    si.on_wait = waits[-1:]
                    pos = insts.index(ins)
                    for k, wt in enumerate(waits[:-1]):
                        d2 = _copy.deepcopy(ins)
                        d2.name = f"{ins.name}-split{k}"
                        d2.sync_info = type(si)(on_wait=[wt], on_update=[])
                        insts.insert(pos + k, d2)
                    break

    return nc


def _get_nc():
    if "nc" not in _NC_CACHE:
        _NC_CACHE["nc"] = build_nc()
    return _NC_CACHE["nc"]


def run_device(dist8, labf8, trace=False):
    """dist8: [8, W*Q, NPROT*FI] f32 device layout; labf8: [8, P] f32 labels-1."""
    nc = _get_nc()
    cls = np.broadcast_to(np.arange(C, dtype=np.float32)[None, :], (Q, C))
    in_maps = []
    for b in range(B):
        labcls = np.concatenate([labf8[b].reshape(Q, 512), cls], axis=1)
        in_maps.append(
            {
                "dist": dist8[b],
                "labcls": np.ascontiguousarray(labcls).astype(ml_dtypes.bfloat16),
            }
        )
    return run_bass_kernel_spmd(nc, in_maps, list(range(B)), trace=trace)


def kernel(
    prototype_distances,
    target_labels,
    proto_class,
    pair_i,
    pair_j,
    pair_cls,
    _trace=False,
    _results_out=None,
):
    dist = np.asarray(prototype_distances, dtype=np.float32).reshape(B, NPROT, P)
    labels = np.asarray(target_labels).reshape(B, P).astype(np.int64)
    proto_class = np.asarray(proto_class, dtype=np.int64)
    pair_i = np.asarray(pair_i, dtype=np.int64)
    pair_j = np.asarray(pair_j, dtype=np.int64)
    pair_cls = np.asarray(pair_cls, dtype=np.int64)

    # Permute prototypes to class-major layout: slot n -> class n // 8.
    perm = np.empty(NPROT, dtype=np.int64)
    for c in range(C):
        protos = np.nonzero(proto_class == c)[0]
        assert len(protos) == 8, "expect 8 prototypes per class"
        perm[8 * c : 8 * c + 8] = protos
    inv = np.empty(NPROT, dtype=np.int64)
    inv[perm] = np.arange(NPROT)

    # Device layout [w, quarter, q, n, i]: pixel p = 512*q + 128*w + 32*k + i,
    # protos class-major.  One transpose+copy host-side buys fully
    # contiguous quarter-window device DMAs.
    QW_ = FI // 4
    dist_v = dist[:, perm, :].reshape(B, NPROT, Q, W, 4, QW_)
    dist_p = np.ascontiguousarray(dist_v.transpose(0, 3, 4, 2, 5, 1)).reshape(
        B, W * 4 * Q, NPROT * QW_
    )
    labf = np.ascontiguousarray((labels - 1).astype(np.float32))

    br = run_device(dist_p, labf, trace=_trace)
    if _results_out is not None:
        _results_out.append(br)

    total_vals = np.float64(0.0)
    total_valid = 0
    for b in range(B):
        out = br.results[b]["g"]  # [81, 80]; out[j, a] = G[a, j], out[80, a] = Z_a
        Z = out[NPROT].astype(np.float64)
        Gt = out[:NPROT].astype(np.float64)  # Gt[j, a] = sum_p em_a * s_j
        with np.errstate(divide="ignore", invalid="ignore"):
            A = np.where(Z[None, :] != 0.0, Gt / Z[None, :], 0.0)  # A[j, a] = E_a[d_j]
        lb = labels[b] - 1
        cnt = np.bincount(lb[lb >= 0], minlength=C)
        ii = inv[pair_i]
        jj = inv[pair_j]
        # A[x, a] = expectation of d_x under softmax of proto a
        kld = 0.5 * (A[jj, jj] - A[jj, ii] + A[ii, ii] - A[ii, jj])
        valid = cnt[pair_cls] >= 2
        total_vals += np.exp(-kld[valid]).sum()
        total_valid += int(valid.sum())

    if total_valid > 0:
        res = np.float32(total_vals / max(total_valid, 1))
    else:
        res = np.float32(0.0)
    return res


if __name__ == "__main__":
    rng = np.random.default_rng(0)
    d = rng.standard_normal((B, NPROT, 256, 256), dtype=np.float32)
    l = rng.integers(0, 11, (B, 256, 256))
    pc = (np.arange(NPROT) % 40) // 4
    pairs = []
    for s in range(2):
        for c in range(C):
            base = s * 40 + c * 4
            for a in range(4):
                for b2 in range(a + 1, 4):
                    pairs.append((base + a, base + b2, c))
    pairs = np.asarray(pairs, np.int32)
    print(kernel(d, l, pc, pairs[:, 0], pairs[:, 1], pairs[:, 2]))


# revision 53
# speedup vs baseline: 1.3334x; 1.1035x over previous
"""Trainium2 Bass kernel for nn_KLDLoss_18769007083961.

Math reformulation (validated vs reference, rel err ~3e-5 with bf16):
  For each image b, prototype a with class c(a), define over pixels p:
    s_a[p]  = d_a[p] + (label[p] != c(a)) * (-1e4)      # masked-biased distance
    em_a[p] = exp(s_a[p])                               # exactly 0 off-class (underflow)
    Z_a     = sum_p em_a[p]
    G[a,j]  = sum_p em_a[p] * s_j[p]   (j in same group => same class mask)
    A[a,j]  = G[a,j] / Z_a
  Symmetric KL for a same-group pair (i,j) (log-partition terms cancel):
    kld = 0.5 * (A[j,j] - A[j,i] + A[i,i] - A[i,j])
  loss = mean over valid pairs (class count >= 2) of exp(-kld).

Only same-class G entries are consumed, and em is exactly zero off-class,
so the bf16 rounding of the -1e4 bias in s never reaches the result: the
biased s tile is written once in bf16 and feeds both the exp (ACT) and the
matmul lhsT (PE).

Performance structure (one image per NeuronCore, 8 cores, ~90us/image):
  * Tiles are PIXEL-MAJOR [q, i, n]: the matmul stationary s16[:, i, :] is
    then contiguous, which drops the LDW+MM cadence from ~182ns (strided)
    to ~64ns per 128-pixel step; 512 steps accumulate the [81, 80] gram
    (80 protos + ones row for Z) in one PSUM bank.
  * The host pre-transposes dist to [w, quarter, q, i, n] (class-major
    protos, pixel p = 512*q + 128*w + 32*k + i), so every quarter-window
    DMA is one fully contiguous copy: no transposing-descriptor storm.
  * 3 s_t buffers: windows 0-2 are issued upfront from the sync queue
    (all 16 DMA queues); window 3 is prefetched from ACT once window 0's
    STTs release its buffer.
  * Per window: DVE builds (label != c) and the bf16 biased s16; ACT does
    exp into em; PE runs 128 matmuls.  Windows are processed in 4 (first/
    last) or 2 (middle) column chunks so the first matmul starts ~13us in
    and the post-DMA tail stays short.
  * Engines have a single sync-wait slot per instruction, so every
    cross-engine dependency is carried by a dedicated 1-element absorber
    op (probe/probe2/obs2/dead3/dead4/dmaobs), pinned ahead of its
    consumer with no-sync dep edges; the kernel compiles with zero
    multi-wait instructions.
  Host does the tiny 120-pair combine on the returned [81, 80] grams.
"""

import sys
from contextlib import ExitStack

import ml_dtypes
import numpy as np

sys.path.insert(0, "/opt/trn_rl_repo")

import concourse.bass as bass
import concourse.tile as tile
from concourse import mybir
from concourse.bass_utils import run_bass_kernel_spmd
from concourse.tile import add_dep_helper

B = 8
C = 10
NPROT = 80
NSLOT = NPROT + 1  # 80 protos + ones column for Z
P = 65536
Q = 128          # partitions = coarse pixel blocks of 512
W = 4            # windows per image
FI = 128         # inner pixels per window per partition
F32 = mybir.dt.float32
BF16 = mybir.dt.bfloat16

_NC_CACHE = {}


def build_nc():
    nc = bass.Bass()
    # dist pre-transposed host-side to [w, quarter, q, n, i]: every quarter-
    # window DMA is a plain contiguous copy, so the first STT chunk starts
    # after ~1/16 of the image has landed.
    d_in = nc.dram_tensor(
        "dist", [W * 4 * Q, NPROT * (FI // 4)], F32, kind="ExternalInput"
    )
    # labels [q, 512] packed with the 10 class constants -> cols 512..521
    lab_in = nc.dram_tensor("labcls", [Q, 512 + C], BF16, kind="ExternalInput")
    g_out = nc.dram_tensor("g", [NSLOT, NPROT], F32, kind="ExternalOutput")

    with ExitStack() as ctx:
        tc = ctx.enter_context(tile.TileContext(nc))
        singles = ctx.enter_context(tc.tile_pool(name="singles", bufs=1))
        spool = ctx.enter_context(tc.tile_pool(name="spool", bufs=3))
        s16pool = ctx.enter_context(tc.tile_pool(name="s16pool", bufs=2))
        empool = ctx.enter_context(tc.tile_pool(name="empool", bufs=2))
        mpool = ctx.enter_context(tc.tile_pool(name="mpool", bufs=2))
        psum = ctx.enter_context(tc.tile_pool(name="psum", bufs=1, space="PSUM"))

        labels_t = singles.tile([Q, 512 + C], BF16)
        nc.sync.dma_start(out=labels_t, in_=lab_in[:, :])
        cls_t = labels_t[:, 512 : 512 + C]

        g_ps = psum.tile([NSLOT, NPROT], F32)

        QW_ = FI // 4  # pixels per quarter-window
        QB = NPROT * QW_  # sbuf columns per quarter block
        s_tiles = []
        for w in range(3):
            s_w = spool.tile([Q, NPROT * FI], F32, tag="s", name=f"s_t{w}")
            for k in range(4):
                nc.sync.dma_start(
                    out=s_w[:, k * QB : (k + 1) * QB],
                    in_=d_in[(4 * w + k) * Q : (4 * w + k + 1) * Q, :],
                )
            s_tiles.append(s_w)


        # Engines have a single sync-wait slot per instruction.  Every
        # cross-engine dependency is therefore carried by a dedicated
        # 1-element absorber op, pinned ahead of its consumer with no-sync
        # dep edges so the scheduler keeps the elision-enabling order.
        first = True
        em_tiles = []
        dead4_tiles = []
        dead4_insts = []
        for w in range(W):
            s_t = s_tiles[w]

            # mne[p, c, i] = (labels != c) as 1.0/0.0, bf16
            mne = mpool.tile([Q, C * FI], BF16, tag="mne")
            mne_v = mne.rearrange("p (c i) -> p c i", c=C)
            lab_w = labels_t[:, w * FI : (w + 1) * FI]
            nc.vector.tensor_tensor(
                mne_v,
                lab_w.unsqueeze(1).broadcast_to([Q, C, FI]),
                cls_t.unsqueeze(2).broadcast_to([Q, C, FI]),
                mybir.AluOpType.not_equal,
            )

            # DVE absorber chain: (1) dist-DMA completion for this window
            probe = mpool.tile([Q, 1], F32, tag="probe", bufs=4)
            i_probe = nc.vector.tensor_copy(probe, s_t[:, 0:1])
            dve_prev = i_probe
            if w >= 2:
                # (2) ACT finished exp(w-2) (read byte from its LAST chunk),
                # which read the s16 buffer the STTs below recycle
                probe2 = mpool.tile([Q, 1], BF16, tag="probe2", bufs=4)
                i_probe2 = nc.vector.tensor_copy(
                    probe2,
                    em_tiles[w - 2][:, (FI - 1) * NPROT : (FI - 1) * NPROT + 1],
                )
                add_dep_helper(i_probe2.ins, dve_prev.ins, sync=False)
                dve_prev = i_probe2

            s16 = s16pool.tile([Q, NSLOT * FI], BF16, tag="s16")
            s16_v = s16.rearrange("p (i n) -> p i n", n=NSLOT)
            em = empool.tile([Q, NPROT * FI], BF16, tag="em")
            em_v = em.rearrange("p (i n) -> p i n", n=NPROT)
            em_tiles.append(em)

            # ones column (slot 80) -> Z row of the gram.  For w >= 2 its
            # bytes were read by every LDW of window w-2, so this memset
            # carries exactly the "PE done with window w-2" wait that the
            # STTs would otherwise each need.
            i_memset = nc.vector.memset(s16_v[:, :, NPROT], 1.0)
            add_dep_helper(i_memset.ins, dve_prev.ins, sync=False)
            dve_prev = i_memset

            # ACT absorber: reading an old-em byte absorbs the same-engine
            # WAW tick (exp(w) overwrites exp(w-2)'s output); the PE tick was
            # absorbed by dead_act at the end of window w-2.
            act_abs = None
            if w >= 2:
                # Read a byte exp(w-2)'s LAST chunk wrote: the single wait
                # "ACT >= exp(w-2, k3)" dominates every chunk's WAW below.
                dead3 = mpool.tile([Q, 1], BF16, tag="dead3", bufs=2)
                act_abs = nc.scalar.copy(
                    dead3,
                    em_tiles[w - 2][
                        :, (FI - 1) * NPROT + 1 : (FI - 1) * NPROT + 2
                    ],
                )
                add_dep_helper(act_abs.ins, dead4_insts[w - 2].ins, sync=False)

            # Chunked pipeline: fine chunks on the first/last window for a
            # fast start/short tail, coarser in the middle to cut DVE
            # per-instruction overhead.  The quarter blocks are contiguous,
            # so s_t is simply a pixel-major [Q, FI, NPROT] tensor.
            s_pm = s_t.rearrange("p (i n) -> p i n", n=NPROT)
            nchunk = 4 if w in (0, W - 1) else 2
            cw = FI // nchunk
            s_next = None
            for k in range(nchunk):
                i0 = k * cw
                if w in (1, 2) and k == 1:
                    # chunk 1 reads quarters 2+3; absorb quarter 2's DMAHW
                    # tick so the first STT below carries only quarter 3's
                    probeb = mpool.tile([Q, 1], F32, tag="probeb", bufs=4)
                    i_pb = nc.vector.tensor_copy(
                        probeb, s_t[:, 2 * QB : 2 * QB + 1]
                    )
                    add_dep_helper(i_pb.ins, dve_prev.ins, sync=False)
                    dve_prev = i_pb
                # s16 = (mne * -1e4) + d, bf16 out, 8 protos per class block
                for c in range(C):
                    n0 = 8 * c
                    mne_b = (
                        mne_v[:, c, i0 : i0 + cw]
                        .unsqueeze(2)
                        .broadcast_to([Q, cw, 8])
                    )
                    i_stt = nc.vector.scalar_tensor_tensor(
                        s16_v[:, i0 : i0 + cw, n0 : n0 + 8],
                        mne_b,
                        -1.0e4,
                        s_pm[:, i0 : i0 + cw, n0 : n0 + 8],
                        mybir.AluOpType.mult,
                        mybir.AluOpType.add,
                    )
                    if c == 0:
                        add_dep_helper(i_stt.ins, dve_prev.ins, sync=False)
                    i_stt_last = i_stt

                act_prev = act_abs
                act_abs = None
                # ACT-side observer of the last STT of this chunk: the exp
                # below then sheds its DVE wait, and the prefetch DMA can
                # issue waitlessly right here.
                obs2 = mpool.tile([Q, 1], BF16, tag="obs2", bufs=8)
                i_obs2 = nc.scalar.copy(
                    obs2,
                    s16[:, (i0 + cw - 1) * NSLOT + 79 : (i0 + cw - 1) * NSLOT + 80],
                )
                if act_prev is not None:
                    add_dep_helper(i_obs2.ins, act_prev.ins, sync=False)
                act_prev = i_obs2
                if k == nchunk - 1 and w + 3 < W:
                    # All of window 0's STTs are done; fetch window 3 into
                    # the freed buffer from the SYNC queue so the transfer
                    # spreads across all 16 DMA queues.  The issue's WAR +
                    # WAW waits exceed the 1-wait ISA slot; the drain-split
                    # pass below converts them into a chain of single-wait
                    # drains on the SP queue.
                    din_v = d_in.rearrange("(a q) m -> a q m", q=Q)
                    s_next = spool.tile(
                        [Q, NPROT * FI], F32, tag="s", name=f"s_t{w+3}"
                    )
                    s_tiles.append(s_next)
                    a0 = 4 * (w + 3)
                    for h2 in range(2):
                        nc.sync.dma_start(
                            out=s_next[
                                :, 2 * h2 * QB : 2 * (h2 + 1) * QB
                            ].rearrange("p (x m) -> p x m", x=2),
                            in_=din_v[a0 + 2 * h2 : a0 + 2 * h2 + 2].transpose(
                                [1, 0, 2]
                            ),
                        )

                # em = exp(s16), bf16
                i_exp = nc.scalar.activation(
                    em_v[:, i0 : i0 + cw, :],
                    s16_v[:, i0 : i0 + cw, :NPROT],
                    mybir.ActivationFunctionType.Exp,
                )
                if act_prev is not None:
                    add_dep_helper(i_exp.ins, act_prev.ins, sync=False)

                for i in range(i0, i0 + cw):
                    nc.tensor.matmul(
                        g_ps,
                        s16_v[:, i, :],
                        em_v[:, i, :],
                        start=first,
                        stop=(w == W - 1 and i == FI - 1),
                    )
                    first = False

            if w + 2 < W:
                # Read the accumulator right after this window's last matmul:
                # the copy waits exactly on "PE done with window w", putting
                # that tick into ACT's clock for window w+2's exp.
                dead4 = mpool.tile([1, 1], F32, tag="dead4", bufs=2)
                dead4_insts.append(nc.scalar.copy(dead4, g_ps[0:1, 0:1]))
                dead4_tiles.append(dead4)

        # Absorb the ACT-PSUM-read serialization into DVE so the final
        # PSUM->SBUF copy carries only the PE wait.
        deadf = mpool.tile([1, 1], F32, tag="deadf", bufs=1)
        i_deadf = nc.vector.tensor_copy(deadf, dead4_tiles[-1])
        g_sb = singles.tile([NSLOT, NPROT], F32)
        i_gcopy = nc.vector.tensor_copy(g_sb, g_ps)
        add_dep_helper(i_gcopy.ins, i_deadf.ins, sync=False)
        # Output DMA from ACT behind a g_sb observer, so the issue carries
        # at most the DMAHW semaphore-recycling wait.
        gobs = mpool.tile([1, 1], F32, tag="gobs", bufs=1)
        i_gobs = nc.scalar.copy(gobs, g_sb[0:1, 0:1])
        add_dep_helper(i_gobs.ins, i_gcopy.ins, sync=False)
        i_gdma = nc.scalar.dma_start(out=g_out[:, :], in_=g_sb)
        add_dep_helper(i_gdma.ins, i_gobs.ins, sync=False)

    # Engine instruction structs hold a single sync wait.  Split EVERY
    # multi-wait instruction (kernel-tail drains, the sync-issued window-3
    # prefetch, ...) into a chain of single-wait drains on the same queue
    # followed by the instruction carrying the final wait.
    import copy as _copy

    drain_template = None
    for fn in nc.m.functions:
        for blk in fn.blocks:
            for ins in blk.instructions:
                if type(ins).__name__ == "InstDrain":
                    drain_template = ins
                    break
            if drain_template is not None:
                break
        if drain_template is not None:
            break

    for fn in nc.m.functions:
        for blk in fn.blocks:
            insts = blk.instructions
            changed = True
            while changed:
                changed = False
                for ins in list(insts):
                    si = ins.sync_info
                    if (
                        si
                        and len(si.on_wait) > 1
                        and not ins.name.endswith("-wsplit-done")
                    ):
                        waits = list(si.on_wait)
                        si.on_wait = waits[-1:]
                        ins.name = f"{ins.name}-wsplit-done"
                        pos = insts.index(ins)
                        for k2, wt in enumerate(waits[:-1]):
                            d2 = _copy.deepcopy(drain_template)
                            d2.name = f"{ins.name}-w{k2}"
                            d2.engine = ins.engine
                            d2.sync_info = type(si)(on_wait=[wt], on_update=[])
                            insts.insert(pos + k2, d2)
                        changed = True
                        break

    return nc


def _get_nc():
    if "nc" not in _NC_CACHE:
        _NC_CACHE["nc"] = build_nc()
    return _NC_CACHE["nc"]


def run_device(dist8, labf8, trace=False):
    """dist8: [8, W*Q, NPROT*FI] f32 device layout; labf8: [8, P] f32 labels-1."""
    nc = _get_nc()
    cls = np.broadcast_to(np.arange(C, dtype=np.float32)[None, :], (Q, C))
    in_maps = []
    for b in range(B):
        labcls = np.concatenate([labf8[b].reshape(Q, 512), cls], axis=1)
        in_maps.append(
            {
                "dist": dist8[b],
                "labcls": np.ascontiguousarray(labcls).astype(ml_dtypes.bfloat16),
            }
        )
    return run_bass_kernel_spmd(nc, in_maps, list(range(B)), trace=trace)


def kernel(
    prototype_distances,
    target_labels,
    proto_class,
    pair_i,
    pair_j,
    pair_cls,
    _trace=False,
    _results_out=None,
):
    dist = np.asarray(prototype_distances, dtype=np.float32).reshape(B, NPROT, P)
    labels = np.asarray(target_labels).reshape(B, P).astype(np.int64)
    proto_class = np.asarray(proto_class, dtype=np.int64)
    pair_i = np.asarray(pair_i, dtype=np.int64)
    pair_j = np.asarray(pair_j, dtype=np.int64)
    pair_cls = np.asarray(pair_cls, dtype=np.int64)

    # Permute prototypes to class-major layout: slot n -> class n // 8.
    perm = np.empty(NPROT, dtype=np.int64)
    for c in range(C):
        protos = np.nonzero(proto_class == c)[0]
        assert len(protos) == 8, "expect 8 prototypes per class"
        perm[8 * c : 8 * c + 8] = protos
    inv = np.empty(NPROT, dtype=np.int64)
    inv[perm] = np.arange(NPROT)

    # Device layout [w, quarter, q, n, i]: pixel p = 512*q + 128*w + 32*k + i,
    # protos class-major.  One transpose+copy host-side buys fully
    # contiguous quarter-window device DMAs.
    QW_ = FI // 4
    dist_v = dist[:, perm, :].reshape(B, NPROT, Q, W, 4, QW_)
    dist_p = np.ascontiguousarray(dist_v.transpose(0, 3, 4, 2, 5, 1)).reshape(
        B, W * 4 * Q, NPROT * QW_
    )
    labf = np.ascontiguousarray((labels - 1).astype(np.float32))

    br = run_device(dist_p, labf, trace=_trace)
    if _results_out is not None:
        _results_out.append(br)

    total_vals = np.float64(0.0)
    total_valid = 0
    for b in range(B):
        out = br.results[b]["g"]  # [81, 80]; out[j, a] = G[a, j], out[80, a] = Z_a
        Z = out[NPROT].astype(np.float64)
        Gt = out[:NPROT].astype(np.float64)  # Gt[j, a] = sum_p em_a * s_j
        with np.errstate(divide="ignore", invalid="ignore"):
            A = np.where(Z[None, :] != 0.0, Gt / Z[None, :], 0.0)  # A[j, a] = E_a[d_j]
        lb = labels[b] - 1
        cnt = np.bincount(lb[lb >= 0], minlength=C)
        ii = inv[pair_i]
        jj = inv[pair_j]
        # A[x, a] = expectation of d_x under softmax of proto a
        kld = 0.5 * (A[jj, jj] - A[jj, ii] + A[ii, ii] - A[ii, jj])
        valid = cnt[pair_cls] >= 2
        total_vals += np.exp(-kld[valid]).sum()
        total_valid += int(valid.sum())

    if total_valid > 0:
        res = np.float32(total_vals / max(total_valid, 1))
    else:
        res = np.float32(0.0)
    return res


if __name__ == "__main__":
    rng = np.random.default_rng(0)
    d = rng.standard_normal((B, NPROT, 256, 256), dtype=np.float32)
    l = rng.integers(0, 11, (B, 256, 256))
    pc = (np.arange(NPROT) % 40) // 4
    pairs = []
    for s in range(2):
        for c in range(C):
            base = s * 40 + c * 4
            for a in range(4):
                for b2 in range(a + 1, 4):
                    pairs.append((base + a, base + b2, c))
    pairs = np.asarray(pairs, np.int32)
    print(kernel(d, l, pc, pairs[:, 0], pairs[:, 1], pairs[:, 2]))
